# revision 24
# baseline (speedup 1.0000x reference)
"""Bahdanau-style additive attention kernel for 8 TRN2 NeuronCores.

Data-parallel over batch B=64 -> 8 batches per core. Weights replicated.

Per-core math (R = 8*196 = 1568 rows, D2 = 2048, H = 512):
  enc1T[h, b]   = sum_d1 W1[d1, h] * input1[b, d1]            (PE, fp32)
  c[h, b]       = enc1T[h, b] + b1[h] + b2[h]                 (ACT)
  enc2T[h, r]   = sum_d W2[d, h] * x[r, d]                    (PE, bf16)
  tmp[h, r]     = relu(enc2T[h, r] + c[h, b(r)])              (ACT, fused w/ PSUM evac)
  scores[r]     = sum_h tmp[h, r] * Wf[h]                     (PE)
  E[r]          = exp(scores[r]);  s[b] = sum_n E             (ACT, fused accum)
  alpha[r]      = E[r] / s[b(r)]                              (DVE)
  att[b, d]     = sum_n alpha[b*196+n] * x[b*196+n, d]        (PE, alpha stationary)

scores+bf is softmax-invariant, so bf is ignored. Softmax skips the max
subtraction: scores are bounded (|scores| < ~4) by construction
(relu(enc2+enc1) @ Wf with Wf ~ U(+-1/sqrt(512))), so exp() is safe in fp32.

x (input2) and W2/Wf are cast to bf16 on host; accumulation is fp32 in PSUM.
Measured end-to-end rel err vs fp32 reference: ~2.5e-3.
"""

import sys

sys.path.insert(0, "/opt/trn_rl_repo")

import numpy as np
import ml_dtypes

import concourse.bass as bass
import concourse.bacc as bacc_mod
import concourse.tile as tile
from concourse import mybir
from concourse.bass_utils import run_bass_kernel_spmd

BF16 = mybir.dt.bfloat16
F32 = mybir.dt.float32
AF = mybir.ActivationFunctionType

B, N, D1, D2, H = 64, 196, 512, 2048, 512
NCORES = 8
BSH = B // NCORES  # 8 batches per core
R = BSH * N  # 1568 rows per core
NG = 8  # row groups in main loop (1 batch each)
RG = R // NG  # 196 rows = 1 batch per group
RHALF = R // 2  # 784 rows = 4 batches per half (transpose-DMA granule)
ND = D2 // 128  # 16 d-tiles
NHT = H // 128  # 4 h-tiles
ND1 = D1 // 128  # 4 d1-tiles
NCH = D2 // 512  # 4 att output chunks


def build_nc():
    nc = bacc_mod.Bacc()

    # All inputs host-packed into exact SBUF layouts (long contiguous runs
    # per partition -> ~1 descriptor per partition per transfer).
    xT_d = nc.declare_dram_parameter("xT", [128, NG, ND * RG], BF16, isOutput=False)
    in1T_d = nc.declare_dram_parameter("in1T", [128, ND1 * BSH], BF16, isOutput=False)
    W1_d = nc.declare_dram_parameter("W1", [128, ND1 * H], BF16, isOutput=False)
    W2_d = nc.declare_dram_parameter("W2", [128, ND * H], BF16, isOutput=False)
    bias_d = nc.declare_dram_parameter("biases", [128, 2 * NHT], F32, isOutput=False)
    Wf_d = nc.declare_dram_parameter("Wf", [128, NHT], BF16, isOutput=False)
    att_d = nc.declare_dram_parameter("att", [128, NG, ND], F32, isOutput=True)
    al_d = nc.declare_dram_parameter("alpha", [R], F32, isOutput=True)

    from contextlib import ExitStack

    with tile.TileContext(nc) as tc, ExitStack() as ctx:
        pers = ctx.enter_context(tc.tile_pool(name="pers", bufs=1))
        small = ctx.enter_context(tc.tile_pool(name="small", bufs=8))
        evac = ctx.enter_context(tc.tile_pool(name="evac", bufs=4))
        p_mm = ctx.enter_context(tc.tile_pool(name="pmm", bufs=4, space="PSUM"))
        p_sc = ctx.enter_context(tc.tile_pool(name="psc", bufs=2, space="PSUM"))
        p_bc = ctx.enter_context(tc.tile_pool(name="pbc", bufs=2, space="PSUM"))

        # ---- small input DMAs (enc1 path + biases), all pre-packed ----
        in1big = pers.tile([128, ND1, BSH], BF16, tag="in1", name="in1big")
        nc.sync.dma_start(in1big[:], in1T_d[:, :].rearrange("p (t b) -> p t b", t=ND1))
        in1sb = [in1big[:, t, :] for t in range(ND1)]
        W1big = pers.tile([128, ND1, H], BF16, tag="w1", name="W1big")
        nc.sync.dma_start(W1big[:], W1_d[:, :].rearrange("p (t h) -> p t h", t=ND1))
        W1sb = [W1big[:, t, :] for t in range(ND1)]
        biassb = pers.tile([128, 2 * NHT], F32, tag="bias", name="biassb")
        nc.sync.dma_start(biassb[:], bias_d[:, :])
        b1sb = biassb[:, 0:NHT]
        b2sb = biassb[:, NHT : 2 * NHT]
        Wfsb = pers.tile([128, NHT], BF16, tag="wf", name="wfsb")
        nc.sync.dma_start(Wfsb[:], Wf_d[:, :])

        # ---- W2 + x^T, chunk-interleaved so enc2 g0 can start ASAP ----
        W2big = pers.tile([128, ND, H], BF16, tag="w2", name="W2big")
        w2src = W2_d[:, :].rearrange("p (d h) -> p d h", d=ND)
        W2sb = [W2big[:, d, :] for d in range(ND)]
        xTq = [
            pers.tile([128, ND, RG], BF16, tag=f"xTq_{g}", name=f"xTq_{g}")
            for g in range(NG)
        ]
        xsrc = [
            xT_d[:, g, :].rearrange("p (t r) -> p t r", t=ND) for g in range(NG)
        ]
        # W2 d0-7 and xTq0 d0-7 first, then their d8-15 halves, then the rest
        nc.sync.dma_start(W2big[:, 0:8, :], w2src[:, 0:8, :])
        nc.sync.dma_start(xTq[0][:, 0:8, :], xsrc[0][:, 0:8, :])
        nc.sync.dma_start(W2big[:, 8:16, :], w2src[:, 8:16, :])
        nc.sync.dma_start(xTq[0][:, 8:16, :], xsrc[0][:, 8:16, :])
        for g in range(1, NG):
            for q in range(2):
                nc.sync.dma_start(
                    xTq[g][:, 8 * q : 8 * (q + 1), :], xsrc[g][:, 8 * q : 8 * (q + 1), :]
                )

        # ---- constants / persistent buffers ----
        b12 = pers.tile([128, NHT], F32, tag="b12", name="b12")
        nc.vector.tensor_add(b12[:], b1sb[:], b2sb[:])
        ones1 = pers.tile([1, 128], BF16, tag="ones", name="ones1")
        nc.vector.memset(ones1[:], 1.0)
        attT = pers.tile([128, NG, ND], F32, tag="attT", name="attT")

        tmp = [pers.tile([128, R], BF16, tag=f"tmp_{ht}", name=f"tmp_{ht}") for ht in range(NHT)]
        E_sb = pers.tile([1, R], F32, tag="E", name="E_sb")
        alpha_bf = pers.tile([1, R], BF16, tag="alpha_bf", name="alpha_bf")
        alpha_sb = pers.tile([1, R], F32, tag="alpha", name="alpha_sb")

        # ---- enc1: enc1T[h, b] ; c = enc1T + b1 + b2 ----
        c_sb = []
        for ht in range(NHT):
            pe1 = p_mm.tile([128, BSH], F32, tag="mm", name="pe1")
            for t in range(ND1):
                nc.tensor.matmul(
                    pe1[:],
                    lhsT=W1sb[t][:, ht * 128 : (ht + 1) * 128],
                    rhs=in1sb[t][:],
                    start=(t == 0),
                    stop=(t == ND1 - 1),
                )
            cs = pers.tile([128, BSH], F32, tag=f"c_{ht}", name=f"c_{ht}")
            nc.scalar.activation(cs[:], pe1[:], AF.Identity, bias=b12[:, ht : ht + 1])
            c_sb.append(cs)

        # ---- main loop over row groups (2 batches each) ----
        ev_idx = 0
        for g in range(NG):
            # enc2T for this group, all 4 h-tiles
            for ht in range(NHT):
                pe2 = p_mm.tile([128, RG], F32, tag="mm", name="pe2")
                for d in range(ND):
                    nc.tensor.matmul(
                        pe2[:],
                        lhsT=W2sb[d][:, ht * 128 : (ht + 1) * 128],
                        rhs=xTq[g][:, d, :],
                        start=(d == 0),
                        stop=(d == ND - 1),
                    )
                # fused bias + relu epilogue (group == batch g)
                nc.scalar.activation(
                    tmp[ht][:, g * N : (g + 1) * N],
                    pe2[:],
                    AF.Relu,
                    bias=c_sb[ht][:, g : g + 1],
                )
            # scores for this group: [1, RG]
            ps = p_sc.tile([1, RG], F32, tag="sc", name="ps")
            for ht in range(NHT):
                nc.tensor.matmul(
                    ps[:],
                    lhsT=Wfsb[:, ht : ht + 1],
                    rhs=tmp[ht][:, g * RG : g * RG + RG],
                    start=(ht == 0),
                    stop=(ht == NHT - 1),
                )
            # softmax tail (batch g)
            bo = g * N
            sums = small.tile([1, 1], F32, tag="sums", name="sums")
            nc.scalar.activation(
                E_sb[:1, bo : bo + N],
                ps[:1, :],
                AF.Exp,
                accum_out=sums[:],
            )
            rs = small.tile([1, 1], F32, tag="rs", name="rs")
            nc.vector.reciprocal(rs[:], sums[:])
            nc.vector.tensor_scalar_mul(
                alpha_sb[:1, bo : bo + N], E_sb[:1, bo : bo + N], rs[:]
            )
            nc.vector.tensor_copy(alpha_bf[:1, bo : bo + N], alpha_sb[:1, bo : bo + N])
            # broadcast alpha of this group to 128 partitions (PE ones-trick)
            pbc = p_bc.tile([128, RG], F32, tag="bc", name="pbc")
            nc.tensor.matmul(
                pbc[:],
                lhsT=ones1[:],
                rhs=alpha_bf[:1, g * RG : (g + 1) * RG],
                start=True,
                stop=True,
            )
            abc = evac.tile([128, RG], BF16, tag="abc", name="abc")
            nc.scalar.copy(abc[:], pbc[:])
            # att: bf16 multiply (DVE 2x) + reduce, in 2 half-chunks to
            # shorten the serial chain per group
            prod = evac.tile([128, ND, RG], BF16, tag="prod", name="prod")
            abc3 = abc[:].rearrange("p (o n) -> p o n", o=1).broadcast_to((128, ND // 2, RG))
            for q in range(2):
                h0 = (ND // 2) * q
                h1 = (ND // 2) * (q + 1)
                nc.vector.tensor_tensor(
                    out=prod[:, h0:h1, :],
                    in0=xTq[g][:, h0:h1, :],
                    in1=abc3,
                    op=mybir.AluOpType.mult,
                )
                nc.vector.tensor_reduce(
                    out=attT[:, g, h0:h1],
                    in_=prod[:, h0:h1, :],
                    axis=mybir.AxisListType.X,
                    op=mybir.AluOpType.add,
                )
            nc.sync.dma_start(att_d[:, g, :], attT[:, g, :])
        # alpha out
        nc.sync.dma_start(al_d[:].rearrange("(p f) -> p f", p=1), alpha_sb[:])

    nc.finalize()
    return nc


_cache = {}


def _get_nc():
    if "nc" not in _cache:
        _cache["nc"] = build_nc()
    return _cache["nc"]


def make_in_maps(input1, input2, W1, b1, W2, b2, Wf, bf):
    bfl = ml_dtypes.bfloat16
    x_all = np.asarray(input2, dtype=np.float32).astype(bfl)
    # packed weights (shared across cores)
    W2b = np.asarray(W2, np.float32).astype(bfl)
    W2p = np.ascontiguousarray(W2b.reshape(ND, 128, H).transpose(1, 0, 2).reshape(128, ND * H))
    W1b = np.asarray(W1, np.float32).astype(bfl)
    W1p = np.ascontiguousarray(W1b.reshape(ND1, 128, H).transpose(1, 0, 2).reshape(128, ND1 * H))
    Wfb = np.asarray(Wf, np.float32).reshape(H).astype(bfl)
    Wfp = np.ascontiguousarray(Wfb.reshape(NHT, 128).T)
    b1f = np.asarray(b1, np.float32).reshape(NHT, 128).T
    b2f = np.asarray(b2, np.float32).reshape(NHT, 128).T
    biasp = np.ascontiguousarray(np.concatenate([b1f, b2f], axis=1))
    in1 = np.asarray(input1, np.float32)
    in_maps = []
    for c in range(NCORES):
        sh = slice(c * BSH, (c + 1) * BSH)
        x = x_all[sh].reshape(R, D2)
        # xT packed: [128(p), NG(g), ND(t), RG(r)] with value x[g*RG+r, t*128+p]
        xTp = np.ascontiguousarray(
            x.reshape(NG, RG, ND, 128).transpose(3, 0, 2, 1).reshape(128, NG, ND * RG)
        )
        in1p = np.ascontiguousarray(
            in1[sh].T.astype(bfl).reshape(ND1, 128, BSH).transpose(1, 0, 2).reshape(128, ND1 * BSH)
        )
        in_maps.append(
            {
                "xT": xTp,
                "in1T": in1p,
                "W1": W1p,
                "W2": W2p,
                "biases": biasp,
                "Wf": Wfp,
            }
        )
    return in_maps


def kernel(input1, input2, W1, b1, W2, b2, Wf, bf):
    in_maps = make_in_maps(input1, input2, W1, b1, W2, b2, Wf, bf)
    res = run_bass_kernel_spmd(_get_nc(), in_maps, core_ids=list(range(NCORES)))
    att = np.concatenate(
        [
            res.results[c]["att"].transpose(1, 2, 0).reshape(BSH, D2)
            for c in range(NCORES)
        ],
        axis=0,
    )
    alpha = np.concatenate(
        [res.results[c]["alpha"].reshape(BSH, N) for c in range(NCORES)], axis=0
    )
    return att.astype(np.float32), alpha.astype(np.float32)


# revision 25
# speedup vs baseline: 1.1781x; 1.1781x over previous
"""Bahdanau-style additive attention kernel for 8 TRN2 NeuronCores.

Data-parallel over batch B=64 -> 8 batches per core. Weights replicated.

Per-core math (R = 8*196 = 1568 rows, D2 = 2048, H = 512):
  enc1T[h, b]   = sum_d1 W1[d1, h] * input1[b, d1]            (PE, fp32)
  c[h, b]       = enc1T[h, b] + b1[h] + b2[h]                 (ACT)
  enc2T[h, r]   = sum_d W2[d, h] * x[r, d]                    (PE, bf16)
  tmp[h, r]     = relu(enc2T[h, r] + c[h, b(r)])              (ACT, fused w/ PSUM evac)
  scores[r]     = sum_h tmp[h, r] * Wf[h]                     (PE)
  E[r]          = exp(scores[r]);  s[b] = sum_n E             (ACT, fused accum)
  alpha[r]      = E[r] / s[b(r)]                              (DVE)
  att[b, d]     = sum_n alpha[b*196+n] * x[b*196+n, d]        (PE, alpha stationary)

scores+bf is softmax-invariant, so bf is ignored. Softmax skips the max
subtraction: scores are bounded (|scores| < ~4) by construction
(relu(enc2+enc1) @ Wf with Wf ~ U(+-1/sqrt(512))), so exp() is safe in fp32.

x (input2) and W2/Wf are cast to bf16 on host; accumulation is fp32 in PSUM.
Measured end-to-end rel err vs fp32 reference: ~2.5e-3.
"""

import sys

sys.path.insert(0, "/opt/trn_rl_repo")

import numpy as np
import ml_dtypes

import concourse.bass as bass
import concourse.bacc as bacc_mod
import concourse.tile as tile
from concourse import mybir
from concourse.bass_utils import run_bass_kernel_spmd

BF16 = mybir.dt.bfloat16
F32 = mybir.dt.float32
AF = mybir.ActivationFunctionType

B, N, D1, D2, H = 64, 196, 512, 2048, 512
NCORES = 8
BSH = B // NCORES  # 8 batches per core
R = BSH * N  # 1568 rows per core
NG = 8  # row groups in main loop (1 batch each)
RG = R // NG  # 196 rows = 1 batch per group
RHALF = R // 2  # 784 rows = 4 batches per half (transpose-DMA granule)
ND = D2 // 128  # 16 d-tiles
NHT = H // 128  # 4 h-tiles
ND1 = D1 // 128  # 4 d1-tiles
NCH = D2 // 512  # 4 att output chunks


def build_nc():
    nc = bacc_mod.Bacc()

    # All inputs host-packed into exact SBUF layouts (long contiguous runs
    # per partition -> ~1 descriptor per partition per transfer).
    xT_d = nc.declare_dram_parameter("xT", [128, NG, ND * RG], BF16, isOutput=False)
    in1T_d = nc.declare_dram_parameter("in1T", [128, ND1 * BSH], BF16, isOutput=False)
    W1_d = nc.declare_dram_parameter("W1", [128, ND1 * H], BF16, isOutput=False)
    W2_d = nc.declare_dram_parameter("W2", [128, ND * H], BF16, isOutput=False)
    bias_d = nc.declare_dram_parameter("biases", [128, 2 * NHT], F32, isOutput=False)
    Wf_d = nc.declare_dram_parameter("Wf", [128, NHT], BF16, isOutput=False)
    att_d = nc.declare_dram_parameter("att", [128, NG, ND], F32, isOutput=True)
    al_d = nc.declare_dram_parameter("alpha", [R], F32, isOutput=True)

    from contextlib import ExitStack

    with tile.TileContext(nc) as tc, ExitStack() as ctx:
        pers = ctx.enter_context(tc.tile_pool(name="pers", bufs=1))
        small = ctx.enter_context(tc.tile_pool(name="small", bufs=8))
        evac = ctx.enter_context(tc.tile_pool(name="evac", bufs=4))
        p_mm = ctx.enter_context(tc.tile_pool(name="pmm", bufs=4, space="PSUM"))
        p_sc = ctx.enter_context(tc.tile_pool(name="psc", bufs=2, space="PSUM"))
        p_bc = ctx.enter_context(tc.tile_pool(name="pbc", bufs=2, space="PSUM"))

        # ---- small input DMAs (enc1 path + biases), all pre-packed ----
        in1big = pers.tile([128, ND1, BSH], BF16, tag="in1", name="in1big")
        nc.sync.dma_start(in1big[:], in1T_d[:, :].rearrange("p (t b) -> p t b", t=ND1))
        in1sb = [in1big[:, t, :] for t in range(ND1)]
        W1big = pers.tile([128, ND1, H], BF16, tag="w1", name="W1big")
        nc.sync.dma_start(W1big[:], W1_d[:, :].rearrange("p (t h) -> p t h", t=ND1))
        W1sb = [W1big[:, t, :] for t in range(ND1)]
        biassb = pers.tile([128, 2 * NHT], F32, tag="bias", name="biassb")
        nc.sync.dma_start(biassb[:], bias_d[:, :])
        b1sb = biassb[:, 0:NHT]
        b2sb = biassb[:, NHT : 2 * NHT]
        Wfsb = pers.tile([128, NHT], BF16, tag="wf", name="wfsb")
        nc.sync.dma_start(Wfsb[:], Wf_d[:, :])

        # ---- W2 + x^T, chunk-interleaved so enc2 g0 can start ASAP ----
        W2big = pers.tile([128, ND, H], BF16, tag="w2", name="W2big")
        w2src = W2_d[:, :].rearrange("p (d h) -> p d h", d=ND)
        W2sb = [W2big[:, d, :] for d in range(ND)]
        xTq = [
            pers.tile([128, ND, RG], BF16, tag=f"xTq_{g}", name=f"xTq_{g}")
            for g in range(NG)
        ]
        xsrc = [
            xT_d[:, g, :].rearrange("p (t r) -> p t r", t=ND) for g in range(NG)
        ]
        # W2 d0-7 and xTq0 d0-7 first, then their d8-15 halves, then the rest
        nc.sync.dma_start(W2big[:, 0:8, :], w2src[:, 0:8, :])
        nc.sync.dma_start(xTq[0][:, 0:8, :], xsrc[0][:, 0:8, :])
        nc.sync.dma_start(W2big[:, 8:16, :], w2src[:, 8:16, :])
        nc.sync.dma_start(xTq[0][:, 8:16, :], xsrc[0][:, 8:16, :])
        for g in range(1, NG):
            for q in range(2):
                nc.sync.dma_start(
                    xTq[g][:, 8 * q : 8 * (q + 1), :], xsrc[g][:, 8 * q : 8 * (q + 1), :]
                )

        # ---- constants / persistent buffers ----
        b12 = pers.tile([128, NHT], F32, tag="b12", name="b12")
        nc.vector.tensor_add(b12[:], b1sb[:], b2sb[:])
        ones1 = pers.tile([1, 128], BF16, tag="ones", name="ones1")
        nc.vector.memset(ones1[:], 1.0)
        attT = pers.tile([128, NG, ND], F32, tag="attT", name="attT")

        tmp = [pers.tile([128, R], BF16, tag=f"tmp_{ht}", name=f"tmp_{ht}") for ht in range(NHT)]
        E_sb = pers.tile([1, R], F32, tag="E", name="E_sb")
        alpha_bf = pers.tile([1, R], BF16, tag="alpha_bf", name="alpha_bf")
        alpha_sb = pers.tile([1, R], F32, tag="alpha", name="alpha_sb")

        # ---- enc1: enc1T[h, b] ; c = enc1T + b1 + b2 ----
        c_sb = []
        for ht in range(NHT):
            pe1 = p_mm.tile([128, BSH], F32, tag="mm", name="pe1")
            for t in range(ND1):
                nc.tensor.matmul(
                    pe1[:],
                    lhsT=W1sb[t][:, ht * 128 : (ht + 1) * 128],
                    rhs=in1sb[t][:],
                    start=(t == 0),
                    stop=(t == ND1 - 1),
                )
            cs = pers.tile([128, BSH], F32, tag=f"c_{ht}", name=f"c_{ht}")
            nc.scalar.activation(cs[:], pe1[:], AF.Identity, bias=b12[:, ht : ht + 1])
            c_sb.append(cs)

        # ---- main loop over row groups (2 batches each) ----
        ev_idx = 0
        for g in range(NG):
            # enc2T for this group, all 4 h-tiles
            for ht in range(NHT):
                pe2 = p_mm.tile([128, RG], F32, tag="mm", name="pe2")
                for d in range(ND):
                    nc.tensor.matmul(
                        pe2[:],
                        lhsT=W2sb[d][:, ht * 128 : (ht + 1) * 128],
                        rhs=xTq[g][:, d, :],
                        start=(d == 0),
                        stop=(d == ND - 1),
                    )
                # fused bias + relu epilogue (group == batch g)
                nc.scalar.activation(
                    tmp[ht][:, g * N : (g + 1) * N],
                    pe2[:],
                    AF.Relu,
                    bias=c_sb[ht][:, g : g + 1],
                )
            # scores for this group: [1, RG]
            ps = p_sc.tile([1, RG], F32, tag="sc", name="ps")
            for ht in range(NHT):
                nc.tensor.matmul(
                    ps[:],
                    lhsT=Wfsb[:, ht : ht + 1],
                    rhs=tmp[ht][:, g * RG : g * RG + RG],
                    start=(ht == 0),
                    stop=(ht == NHT - 1),
                )
            # softmax tail (batch g)
            bo = g * N
            sums = small.tile([1, 1], F32, tag="sums", name="sums")
            nc.scalar.activation(
                E_sb[:1, bo : bo + N],
                ps[:1, :],
                AF.Exp,
                accum_out=sums[:],
            )
            rs = small.tile([1, 1], F32, tag="rs", name="rs")
            nc.vector.reciprocal(rs[:], sums[:])
            nc.vector.tensor_scalar_mul(
                alpha_sb[:1, bo : bo + N], E_sb[:1, bo : bo + N], rs[:]
            )
            nc.vector.tensor_copy(alpha_bf[:1, bo : bo + N], alpha_sb[:1, bo : bo + N])
            # broadcast alpha of this group to 128 partitions (PE ones-trick)
            pbc = p_bc.tile([128, RG], F32, tag="bc", name="pbc")
            nc.tensor.matmul(
                pbc[:],
                lhsT=ones1[:],
                rhs=alpha_bf[:1, g * RG : (g + 1) * RG],
                start=True,
                stop=True,
            )
            abc = evac.tile([128, RG], BF16, tag="abc", name="abc")
            nc.scalar.copy(abc[:], pbc[:])
            prod = evac.tile([128, ND, RG], BF16, tag="prod", name="prod")
            nc.vector.tensor_tensor(
                out=prod[:],
                in0=xTq[g][:],
                in1=abc[:].rearrange("p (o n) -> p o n", o=1).broadcast_to((128, ND, RG)),
                op=mybir.AluOpType.mult,
            )
            nc.vector.tensor_reduce(
                out=attT[:, g, :],
                in_=prod[:],
                axis=mybir.AxisListType.X,
                op=mybir.AluOpType.add,
            )
            nc.sync.dma_start(att_d[:, g, :], attT[:, g, :])
        # alpha out
        nc.sync.dma_start(al_d[:].rearrange("(p f) -> p f", p=1), alpha_sb[:])

    nc.finalize()
    return nc


_cache = {}


def _get_nc():
    if "nc" not in _cache:
        _cache["nc"] = build_nc()
    return _cache["nc"]


def make_in_maps(input1, input2, W1, b1, W2, b2, Wf, bf):
    bfl = ml_dtypes.bfloat16
    x_all = np.asarray(input2, dtype=np.float32).astype(bfl)
    # packed weights (shared across cores)
    W2b = np.asarray(W2, np.float32).astype(bfl)
    W2p = np.ascontiguousarray(W2b.reshape(ND, 128, H).transpose(1, 0, 2).reshape(128, ND * H))
    W1b = np.asarray(W1, np.float32).astype(bfl)
    W1p = np.ascontiguousarray(W1b.reshape(ND1, 128, H).transpose(1, 0, 2).reshape(128, ND1 * H))
    Wfb = np.asarray(Wf, np.float32).reshape(H).astype(bfl)
    Wfp = np.ascontiguousarray(Wfb.reshape(NHT, 128).T)
    b1f = np.asarray(b1, np.float32).reshape(NHT, 128).T
    b2f = np.asarray(b2, np.float32).reshape(NHT, 128).T
    biasp = np.ascontiguousarray(np.concatenate([b1f, b2f], axis=1))
    in1 = np.asarray(input1, np.float32)
    in_maps = []
    for c in range(NCORES):
        sh = slice(c * BSH, (c + 1) * BSH)
        x = x_all[sh].reshape(R, D2)
        # xT packed: [128(p), NG(g), ND(t), RG(r)] with value x[g*RG+r, t*128+p]
        xTp = np.ascontiguousarray(
            x.reshape(NG, RG, ND, 128).transpose(3, 0, 2, 1).reshape(128, NG, ND * RG)
        )
        in1p = np.ascontiguousarray(
            in1[sh].T.astype(bfl).reshape(ND1, 128, BSH).transpose(1, 0, 2).reshape(128, ND1 * BSH)
        )
        in_maps.append(
            {
                "xT": xTp,
                "in1T": in1p,
                "W1": W1p,
                "W2": W2p,
                "biases": biasp,
                "Wf": Wfp,
            }
        )
    return in_maps


def kernel(input1, input2, W1, b1, W2, b2, Wf, bf):
    in_maps = make_in_maps(input1, input2, W1, b1, W2, b2, Wf, bf)
    res = run_bass_kernel_spmd(_get_nc(), in_maps, core_ids=list(range(NCORES)))
    att = np.concatenate(
        [
            res.results[c]["att"].transpose(1, 2, 0).reshape(BSH, D2)
            for c in range(NCORES)
        ],
        axis=0,
    )
    alpha = np.concatenate(
        [res.results[c]["alpha"].reshape(BSH, N) for c in range(NCORES)], axis=0
    )
    return att.astype(np.float32), alpha.astype(np.float32)
